# revision 19
# baseline (speedup 1.0000x reference)
"""Trainium2 Bass kernel: per-pixel 19x19 batch blur (KPN-style).

Reference computation:
    out[b,c,i,j] = (1/361) * sum_{ki,kj} pad[b,c,i+ki,j+kj] * kern[b, i*W+j, ki, kj]
with `pad` the 9-pixel reflection-padded input, shapes:
    input  (2, 3, 256, 256) f32
    kernel (2, 65536, 19, 19) f32    <- 189 MB, dominates memory traffic

Sharding: pure data parallel over (batch, H-tile): 8 cores = 2 batches x 4
tiles of 64 output rows each.  Each core receives (all bf16; rel-err budget
is 2e-2 and bf16 products only cost ~2e-3):
  - its contiguous kernel slice  (16384, 361) bf16  (11.8 MB)
  - SHIFTC [2, 3, 128, 19*82] bf16 (2.4 MB): for jblk/channel, partition j
    holds the flattened sliding strips  SHIFTC[jb,c,j, 19*r+kj] =
    pad[c, r, jb*128+j+kj].  With this layout the 361-tap patch of output
    row i is CONTIGUOUS at free offset 19*i (since k2 = 19*ki + kj).
and writes a (256, 192) f32 block = [jblk*128+j, c*64+i] that the host
transposes back into (3, 64, 256).

Device algorithm (per core), pixels-on-partitions.  The fused DVE
scalar_tensor_tensor (mult+accum) runs at 1 elem/cycle/lane (no fast perf
mode exists for accum-bearing DVE ops), which makes the Vector engine the
bottleneck (~456 ns/row, measured).  So rows are split across two engines:
  - DVE-STT rows: one fused scalar_tensor_tensor per (row, channel):
        scr = (kern_row * 1/361) * patch_view; accum_out[j] = sum(scr)
  - ACT rows: DVE runs ONE bf16 tensor_tensor multiply per 8-row block
    (2x_1p mode, ~196 ns/row) into a products tile; the otherwise-idle
    Activation engine then reduces each row via activation(Copy,
    scale=1/361, accum_out) (~682 ns/row, measured).
With ACT taking ~23/48 blocks both engines finish together:
  DVE: 25*3648 + 23*1565 ~ 127 us;  ACT: 23*5456 ~ 125 us
vs 175 us DVE-only.
"""

import os
import sys

import numpy as np
import ml_dtypes

for _p in ("/opt/trn_rl_repo", os.path.expanduser("~/.axon_site/_ro/trn_rl_repo")):
    if os.path.isdir(_p) and _p not in sys.path:
        sys.path.insert(0, _p)

from contextlib import ExitStack

from concourse import bacc, bass_utils, mybir, tile
from concourse.ap import AP

# Problem constants (hardcoded per the self-containment contract).
B, C, H, W = 2, 3, 256, 256
L = 19
PAD = L // 2  # 9
K2 = L * L  # 361
N_CORES = 8
ROWS_PER_CORE = H // 4  # 64  (4 H-tiles x 2 batches = 8 cores)
PR = ROWS_PER_CORE + 2 * PAD  # 82 padded rows per core
SFREE = L * PR  # 1558 free elems per SHIFTC partition
R_CHUNK = 8  # output rows per kernel-DMA chunk
N_IBLK = ROWS_PER_CORE // R_CHUNK  # 8
F32 = mybir.dt.float32
BF16 = mybir.dt.bfloat16
BLK_FREE = R_CHUNK * K2  # 2888

# 48 blocks of 8 rows: block t = (jb, iblk, c).  ACT_SHARE of them are
# reduced on the Activation engine (DVE only does the 2x multiply).
N_BLK = N_IBLK * 2 * C  # 48
# Rows reduced on ACT (of 384): measured balance point.  Blocks 0 and 1 are
# hand-scheduled (5 ACT rows each); the rest are spread as full 8-row blocks.
ACT_FULL_BLOCKS = 22
# Last blocks stay on DVE so the ACT pipeline drains before Vector finishes.
ACT_TAIL_PAD = 3

_CACHE: dict = {}


def _act_flags(n_act):
    """Spread n_act True flags over blocks 2..N_BLK-ACT_TAIL_PAD-1 (blocks 0-1
    are the hand-scheduled opening blocks that feed ACT immediately)."""
    dom = N_BLK - ACT_TAIL_PAD - 2
    flags = [False] * N_BLK
    for t in range(dom):
        if ((t + 1) * n_act) // dom > (t * n_act) // dom:
            flags[2 + t] = True
    return flags


def _build_program(n_act=ACT_FULL_BLOCKS):
    nc = bacc.Bacc(
        "TRN2",
        target_bir_lowering=False,
        debug=False,
        enable_asserts=False,
        num_devices=N_CORES,
    )
    kern = nc.dram_tensor("kern", [ROWS_PER_CORE * W, K2], BF16, kind="ExternalInput")
    shiftd = nc.dram_tensor("shiftc", [2, C, 128, SFREE], BF16, kind="ExternalInput")
    outd = nc.dram_tensor("out", [2 * 128, C * ROWS_PER_CORE], F32, kind="ExternalOutput")

    mult = mybir.AluOpType.mult
    act_flag = _act_flags(n_act)  # blocks 0,1 hand-scheduled below

    with tile.TileContext(nc) as tc, ExitStack() as ctx:
        cpool = ctx.enter_context(tc.tile_pool(name="cpool", bufs=1))
        kpool = ctx.enter_context(tc.tile_pool(name="kpool", bufs=3))
        spool = ctx.enter_context(tc.tile_pool(name="spool", bufs=3))
        ppool = ctx.enter_context(tc.tile_pool(name="ppool", bufs=3))
        apool = ctx.enter_context(tc.tile_pool(name="apool", bufs=3))

        shiftc = {}

        def _load_shiftc(c, jb):
            sc = cpool.tile([128, SFREE], BF16, name=f"shiftc_{c}_{jb}")
            shiftc[(c, jb)] = sc
            nc.sync.dma_start(
                out=sc[:, :],
                in_=AP(shiftd, (jb * C + c) * 128 * SFREE, [(SFREE, 128), (1, SFREE)]),
            )

        def _load_rows(iblk, jb, r0, nrows):
            """Load kern rows [iblk*8+r0, iblk*8+r0+nrows) of j-half jb."""
            kt = kpool.tile([128, nrows * K2], BF16, name="kt", tag=f"kt{nrows}")
            base = ((iblk * R_CHUNK + r0) * W + jb * 128) * K2
            nc.sync.dma_start(
                out=AP(kt.tensor, 0, [(nrows * K2, 128), (K2, nrows), (1, K2)]),
                in_=AP(kern, base, [(K2, 128), (W * K2, nrows), (1, K2)]),
            )
            return kt

        # Startup: the first compute needs only the first ~500 columns of
        # shiftc(0,0) + a 3-row kern mini-chunk (~0.4 MB); everything else
        # queues behind them.
        MINI = 3
        SPLIT = (R_CHUNK - 1 + L) * L  # 494 cols cover all 8 row windows
        sc0 = cpool.tile([128, SFREE], BF16, name="shiftc_0_0")
        shiftc[(0, 0)] = sc0
        nc.sync.dma_start(
            out=sc0[:, :SPLIT], in_=AP(shiftd, 0, [(SFREE, 128), (1, SPLIT)])
        )
        kt_mini = _load_rows(0, 0, 0, MINI)
        kt_rest = _load_rows(0, 0, MINI, R_CHUNK - MINI)
        nc.sync.dma_start(
            out=sc0[:, SPLIT:],
            in_=AP(shiftd, SPLIT, [(SFREE, 128), (1, SFREE - SPLIT)]),
        )
        _load_shiftc(1, 0)
        _load_shiftc(2, 0)

        outt = [cpool.tile([128, C * ROWS_PER_CORE], F32, name=f"outt{jb}") for jb in range(2)]

        def _stt(kt, koff, sc, i, jb, col):
            scr = spool.tile([128, K2], BF16, name="scr", tag="scr")
            # scr = (kern_row * 1/361) * patch; accum = sum(scr)
            nc.vector.scalar_tensor_tensor(
                out=scr[:, :],
                in0=AP(kt.tensor, koff * K2, [(kt.shape[1], 128), (1, K2)]),
                scalar=1.0 / K2,
                in1=AP(sc.tensor, i * L, [(SFREE, 128), (1, K2)]),
                op0=mult,
                op1=mult,
                accum_out=outt[jb][:, col : col + 1],
            )

        def _tt_mult(kt, koff, nrows, sc, i0):
            prod = ppool.tile([128, nrows * K2], BF16, name="prod", tag=f"prod{nrows}")
            nc.vector.tensor_tensor(
                out=prod[:, :],
                in0=AP(kt.tensor, koff * K2, [(kt.shape[1], 128), (1, nrows * K2)]),
                in1=AP(sc.tensor, i0 * L, [(SFREE, 128), (L, nrows), (1, K2)]),
                op=mult,
            )
            return prod

        def _act_reduce(prod, il, jb, col):
            ascr = apool.tile([128, K2], BF16, name="ascr", tag="ascr")
            nc.scalar.activation(
                out=ascr[:, :],
                in_=AP(prod.tensor, il * K2, [(prod.shape[1], 128), (1, K2)]),
                func=mybir.ActivationFunctionType.Copy,
                scale=1.0 / K2,
                accum_out=outt[jb][:, col : col + 1],
            )

        def _block(kt, sc, iblk, jb, c, n_act_rows):
            i0 = iblk * R_CHUNK
            col0 = c * ROWS_PER_CORE + i0
            if n_act_rows:
                prod = _tt_mult(kt, 0, n_act_rows, sc, i0)
                for il in range(n_act_rows):
                    _act_reduce(prod, il, jb, col0 + il)
            for il in range(n_act_rows, R_CHUNK):
                _stt(kt, il, sc, i0 + il, jb, col0 + il)

        # --- (jb0, iblk0): hand-scheduled opening --------------------------
        # Block 0 (c0): rows 0-2 via STT off the mini chunk (first compute
        # after only ~0.4 MB of DMA); rows 3-7 are ACT's first work (needs
        # only sc0 + kt_rest, no extra shiftc wait).  Block 1 (c1): same
        # split so ACT stays fed while the pattern warms up.
        sc1, sc2 = shiftc[(1, 0)], shiftc[(2, 0)]
        for il in range(MINI):
            _stt(kt_mini, il, sc0, il, 0, il)
        pr = _tt_mult(kt_rest, 0, R_CHUNK - MINI, sc0, MINI)
        for il in range(R_CHUNK - MINI):
            _act_reduce(pr, il, 0, MINI + il)
        pr1 = _tt_mult(kt_rest, 0, R_CHUNK - MINI, sc1, MINI)
        for il in range(R_CHUNK - MINI):
            _act_reduce(pr1, il, 0, ROWS_PER_CORE + MINI + il)
        for c in range(C):
            _load_shiftc(c, 1)

        def _split_block(sc, c, use_act):
            if use_act:
                pm = _tt_mult(kt_mini, 0, MINI, sc, 0)
                for il in range(MINI):
                    _act_reduce(pm, il, 0, c * ROWS_PER_CORE + il)
                p2 = _tt_mult(kt_rest, 0, R_CHUNK - MINI, sc, MINI)
                for il in range(R_CHUNK - MINI):
                    _act_reduce(p2, il, 0, c * ROWS_PER_CORE + MINI + il)
            else:
                for il in range(MINI):
                    _stt(kt_mini, il, sc, il, 0, c * ROWS_PER_CORE + il)
                for il in range(MINI, R_CHUNK):
                    _stt(kt_rest, il - MINI, sc, il, 0, c * ROWS_PER_CORE + il)

        _split_block(sc2, 2, act_flag[2])
        for il in range(MINI):
            _stt(kt_mini, il, sc1, il, 0, ROWS_PER_CORE + il)

        # Per-block ACT row counts: flagged blocks are full (8), except the
        # last flagged one (6) to fine-trim the engine balance.
        act_n = [8 if f else 0 for f in act_flag]
        act_n[max(t for t in range(N_BLK) if act_flag[t])] = 6

        # --- main loop: jb-major so jb0's output DMA overlaps jb1 compute --
        for jb in range(2):
            for iblk in range(N_IBLK):
                if jb == 0 and iblk == 0:
                    continue
                kt = _load_rows(iblk, jb, 0, R_CHUNK)
                t0 = (jb * N_IBLK + iblk) * C
                # ACT blocks first so the Activation engine stays fed.
                order = sorted(range(C), key=lambda c: not act_flag[t0 + c])
                for c in order:
                    _block(kt, shiftc[(c, jb)], iblk, jb, c, act_n[t0 + c])
                if jb == 1 and iblk == N_IBLK - 2:
                    # columns i<56 of every channel are done; flush them so
                    # only a small tail DMA remains after the last block
                    head = (N_IBLK - 1) * R_CHUNK  # 56
                    nc.sync.dma_start(
                        out=AP(
                            outd,
                            128 * (C * ROWS_PER_CORE),
                            [(C * ROWS_PER_CORE, 128), (ROWS_PER_CORE, C), (1, head)],
                        ),
                        in_=AP(
                            outt[1].tensor,
                            0,
                            [(C * ROWS_PER_CORE, 128), (ROWS_PER_CORE, C), (1, head)],
                        ),
                    )
            if jb == 0:
                nc.sync.dma_start(
                    out=AP(
                        outd,
                        0,
                        [(C * ROWS_PER_CORE, 128), (1, C * ROWS_PER_CORE)],
                    ),
                    in_=outt[0][:, :],
                )
            else:
                head = (N_IBLK - 1) * R_CHUNK
                nc.sync.dma_start(
                    out=AP(
                        outd,
                        128 * (C * ROWS_PER_CORE) + head,
                        [(C * ROWS_PER_CORE, 128), (ROWS_PER_CORE, C), (1, R_CHUNK)],
                    ),
                    in_=AP(
                        outt[1].tensor,
                        head,
                        [(C * ROWS_PER_CORE, 128), (ROWS_PER_CORE, C), (1, R_CHUNK)],
                    ),
                )

    nc.compile()
    return nc


def _program():
    if "nc" not in _CACHE:
        _CACHE["nc"] = _build_program()
    return _CACHE["nc"]


def _shard_inputs(input, kernel):
    inp = np.ascontiguousarray(np.asarray(input, dtype=np.float32))
    kern = np.asarray(kernel, dtype=np.float32)
    pad = np.pad(inp, ((0, 0), (0, 0), (PAD, PAD), (PAD, PAD)), mode="reflect")
    # sliding horizontal strips: strips[b, c, r, j, kj] = pad[b, c, r, j + kj]
    strips = np.lib.stride_tricks.sliding_window_view(pad, L, axis=3)
    in_maps = []
    for core in range(N_CORES):
        b, q = divmod(core, 4)
        r0 = q * ROWS_PER_CORE
        # SHIFTC[jb, c, j, 19*r + kj] = pad[b, c, r0 + r, jb*128 + j + kj]
        s = strips[b, :, r0 : r0 + PR, :, :]  # (C, PR, 256, L)
        s = s.transpose(2, 0, 1, 3).reshape(2, 128, C, PR * L)  # (jb*128+j, c, r*L+kj)
        sc = np.ascontiguousarray(s.transpose(0, 2, 1, 3)).astype(ml_dtypes.bfloat16)
        ks = kern[b].reshape(H * W, K2)[
            q * ROWS_PER_CORE * W : (q + 1) * ROWS_PER_CORE * W
        ].astype(ml_dtypes.bfloat16)
        in_maps.append({"kern": ks, "shiftc": sc})
    return in_maps


def _unshard_output(results):
    out = np.empty((B, C, H, W), dtype=np.float32)
    for core in range(N_CORES):
        b, q = divmod(core, 4)
        arr = np.asarray(results[core]["out"])  # (256, 192) = [jb*128+j, c*64+i]
        blk = arr.reshape(2, 128, C, ROWS_PER_CORE).transpose(2, 3, 0, 1)
        out[b, :, q * ROWS_PER_CORE : (q + 1) * ROWS_PER_CORE, :] = blk.reshape(
            C, ROWS_PER_CORE, W
        )
    return out


def run_sharded(inputs, **kw):
    """Run the compiled SPMD program; returns BassKernelResults (for profiling)."""
    in_maps = _shard_inputs(inputs["input"], inputs["kernel"])
    return bass_utils.run_bass_kernel_spmd(
        _program(), in_maps, core_ids=list(range(N_CORES)), **kw
    )


def kernel(input, kernel):
    res = run_sharded({"input": input, "kernel": kernel})
    return _unshard_output(res.results)


# revision 20
# speedup vs baseline: 1.0096x; 1.0096x over previous
"""Trainium2 Bass kernel: per-pixel 19x19 batch blur (KPN-style).

Reference computation:
    out[b,c,i,j] = (1/361) * sum_{ki,kj} pad[b,c,i+ki,j+kj] * kern[b, i*W+j, ki, kj]
with `pad` the 9-pixel reflection-padded input, shapes:
    input  (2, 3, 256, 256) f32
    kernel (2, 65536, 19, 19) f32    <- 189 MB, dominates memory traffic

Sharding: pure data parallel over (batch, H-tile): 8 cores = 2 batches x 4
tiles of 64 output rows each.  Each core receives (all bf16; rel-err budget
is 2e-2 and bf16 products only cost ~2e-3):
  - its contiguous kernel slice  (16384, 361) bf16  (11.8 MB)
  - SHIFTC [2, 3, 128, 19*82] bf16 (2.4 MB): for jblk/channel, partition j
    holds the flattened sliding strips  SHIFTC[jb,c,j, 19*r+kj] =
    pad[c, r, jb*128+j+kj].  With this layout the 361-tap patch of output
    row i is CONTIGUOUS at free offset 19*i (since k2 = 19*ki + kj).
and writes a (256, 192) f32 block = [jblk*128+j, c*64+i] that the host
transposes back into (3, 64, 256).

Device algorithm (per core), pixels-on-partitions.  The fused DVE
scalar_tensor_tensor (mult+accum) runs at 1 elem/cycle/lane (no fast perf
mode exists for accum-bearing DVE ops), which makes the Vector engine the
bottleneck (~456 ns/row, measured).  So rows are split across two engines:
  - DVE-STT rows: one fused scalar_tensor_tensor per (row, channel):
        scr = (kern_row * 1/361) * patch_view; accum_out[j] = sum(scr)
  - ACT rows: DVE runs ONE bf16 tensor_tensor multiply per 8-row block
    (2x_1p mode, ~196 ns/row) into a products tile; the otherwise-idle
    Activation engine then reduces each row via activation(Copy,
    scale=1/361, accum_out) (~682 ns/row, measured).
With ACT taking ~23/48 blocks both engines finish together:
  DVE: 25*3648 + 23*1565 ~ 127 us;  ACT: 23*5456 ~ 125 us
vs 175 us DVE-only.
"""

import os
import sys

import numpy as np
import ml_dtypes

for _p in ("/opt/trn_rl_repo", os.path.expanduser("~/.axon_site/_ro/trn_rl_repo")):
    if os.path.isdir(_p) and _p not in sys.path:
        sys.path.insert(0, _p)

from contextlib import ExitStack

from concourse import bacc, bass_utils, mybir, tile
from concourse.ap import AP

# Problem constants (hardcoded per the self-containment contract).
B, C, H, W = 2, 3, 256, 256
L = 19
PAD = L // 2  # 9
K2 = L * L  # 361
N_CORES = 8
ROWS_PER_CORE = H // 4  # 64  (4 H-tiles x 2 batches = 8 cores)
PR = ROWS_PER_CORE + 2 * PAD  # 82 padded rows per core
SFREE = L * PR  # 1558 free elems per SHIFTC partition
R_CHUNK = 8  # output rows per kernel-DMA chunk
N_IBLK = ROWS_PER_CORE // R_CHUNK  # 8
F32 = mybir.dt.float32
BF16 = mybir.dt.bfloat16
BLK_FREE = R_CHUNK * K2  # 2888

# 48 blocks of 8 rows: block t = (jb, iblk, c).  ACT_SHARE of them are
# reduced on the Activation engine (DVE only does the 2x multiply).
N_BLK = N_IBLK * 2 * C  # 48
# Rows reduced on ACT (of 384): measured balance point.  Blocks 0 and 1 are
# hand-scheduled (5 ACT rows each); the rest are spread as full 8-row blocks.
ACT_FULL_BLOCKS = 22
# Last blocks stay on DVE so the ACT pipeline drains before Vector finishes.
ACT_TAIL_PAD = 3

_CACHE: dict = {}


def _act_flags(n_act):
    """Spread n_act True flags over blocks 2..N_BLK-ACT_TAIL_PAD-1 (blocks 0-1
    are the hand-scheduled opening blocks that feed ACT immediately)."""
    dom = N_BLK - ACT_TAIL_PAD - 2
    flags = [False] * N_BLK
    for t in range(dom):
        if ((t + 1) * n_act) // dom > (t * n_act) // dom:
            flags[2 + t] = True
    return flags


def _build_program(n_act=ACT_FULL_BLOCKS):
    nc = bacc.Bacc(
        "TRN2",
        target_bir_lowering=False,
        debug=False,
        enable_asserts=False,
        num_devices=N_CORES,
    )
    kern = nc.dram_tensor("kern", [ROWS_PER_CORE * W, K2], BF16, kind="ExternalInput")
    shiftd = nc.dram_tensor("shiftc", [2, C, 128, SFREE], BF16, kind="ExternalInput")
    outd = nc.dram_tensor("out", [2 * 128, C * ROWS_PER_CORE], F32, kind="ExternalOutput")

    mult = mybir.AluOpType.mult
    act_flag = _act_flags(n_act)  # blocks 0,1 hand-scheduled below

    with tile.TileContext(nc) as tc, ExitStack() as ctx:
        cpool = ctx.enter_context(tc.tile_pool(name="cpool", bufs=1))
        kpool = ctx.enter_context(tc.tile_pool(name="kpool", bufs=3))
        spool = ctx.enter_context(tc.tile_pool(name="spool", bufs=3))
        ppool = ctx.enter_context(tc.tile_pool(name="ppool", bufs=3))
        apool = ctx.enter_context(tc.tile_pool(name="apool", bufs=3))

        shiftc = {}

        def _load_shiftc(c, jb):
            sc = cpool.tile([128, SFREE], BF16, name=f"shiftc_{c}_{jb}")
            shiftc[(c, jb)] = sc
            nc.sync.dma_start(
                out=sc[:, :],
                in_=AP(shiftd, (jb * C + c) * 128 * SFREE, [(SFREE, 128), (1, SFREE)]),
            )

        def _load_rows(iblk, jb, r0, nrows):
            """Load kern rows [iblk*8+r0, iblk*8+r0+nrows) of j-half jb."""
            kt = kpool.tile([128, nrows * K2], BF16, name="kt", tag=f"kt{nrows}")
            base = ((iblk * R_CHUNK + r0) * W + jb * 128) * K2
            nc.sync.dma_start(
                out=AP(kt.tensor, 0, [(nrows * K2, 128), (K2, nrows), (1, K2)]),
                in_=AP(kern, base, [(K2, 128), (W * K2, nrows), (1, K2)]),
            )
            return kt

        # Startup: the first compute needs only the first ~500 columns of
        # shiftc(0,0) + a 3-row kern mini-chunk (~0.4 MB); everything else
        # queues behind them.
        MINI = 3
        SPLIT = (R_CHUNK - 1 + L) * L  # 494 cols cover all 8 row windows
        sc0 = cpool.tile([128, SFREE], BF16, name="shiftc_0_0")
        shiftc[(0, 0)] = sc0
        nc.sync.dma_start(
            out=sc0[:, :SPLIT], in_=AP(shiftd, 0, [(SFREE, 128), (1, SPLIT)])
        )
        kt_mini = _load_rows(0, 0, 0, MINI)
        kt_rest = _load_rows(0, 0, MINI, R_CHUNK - MINI)
        nc.sync.dma_start(
            out=sc0[:, SPLIT:],
            in_=AP(shiftd, SPLIT, [(SFREE, 128), (1, SFREE - SPLIT)]),
        )
        _load_shiftc(1, 0)
        _load_shiftc(2, 0)

        outt = [cpool.tile([128, C * ROWS_PER_CORE], F32, name=f"outt{jb}") for jb in range(2)]

        def _stt(kt, koff, sc, i, jb, col):
            scr = spool.tile([128, K2], BF16, name="scr", tag="scr")
            # scr = (kern_row * 1/361) * patch; accum = sum(scr)
            nc.vector.scalar_tensor_tensor(
                out=scr[:, :],
                in0=AP(kt.tensor, koff * K2, [(kt.shape[1], 128), (1, K2)]),
                scalar=1.0 / K2,
                in1=AP(sc.tensor, i * L, [(SFREE, 128), (1, K2)]),
                op0=mult,
                op1=mult,
                accum_out=outt[jb][:, col : col + 1],
            )

        def _tt_mult(kt, koff, nrows, sc, i0):
            prod = ppool.tile([128, nrows * K2], BF16, name="prod", tag=f"prod{nrows}")
            nc.vector.tensor_tensor(
                out=prod[:, :],
                in0=AP(kt.tensor, koff * K2, [(kt.shape[1], 128), (1, nrows * K2)]),
                in1=AP(sc.tensor, i0 * L, [(SFREE, 128), (L, nrows), (1, K2)]),
                op=mult,
            )
            return prod

        def _act_reduce(prod, il, jb, col):
            ascr = apool.tile([128, K2], BF16, name="ascr", tag="ascr")
            nc.scalar.activation(
                out=ascr[:, :],
                in_=AP(prod.tensor, il * K2, [(prod.shape[1], 128), (1, K2)]),
                func=mybir.ActivationFunctionType.Copy,
                scale=1.0 / K2,
                accum_out=outt[jb][:, col : col + 1],
            )

        def _block(kt, sc, iblk, jb, c, n_act_rows):
            i0 = iblk * R_CHUNK
            col0 = c * ROWS_PER_CORE + i0
            if n_act_rows:
                prod = _tt_mult(kt, 0, n_act_rows, sc, i0)
                for il in range(n_act_rows):
                    _act_reduce(prod, il, jb, col0 + il)
            for il in range(n_act_rows, R_CHUNK):
                _stt(kt, il, sc, i0 + il, jb, col0 + il)

        # --- (jb0, iblk0): hand-scheduled opening --------------------------
        # Block 0 (c0): rows 0-2 via STT off the mini chunk (first compute
        # after only ~0.4 MB of DMA); rows 3-7 are ACT's first work (needs
        # only sc0 + kt_rest, no extra shiftc wait).  Block 1 (c1): same
        # split so ACT stays fed while the pattern warms up.
        sc1, sc2 = shiftc[(1, 0)], shiftc[(2, 0)]
        for il in range(MINI):
            _stt(kt_mini, il, sc0, il, 0, il)
        pr = _tt_mult(kt_rest, 0, R_CHUNK - MINI, sc0, MINI)
        for il in range(R_CHUNK - MINI):
            _act_reduce(pr, il, 0, MINI + il)
        pr1 = _tt_mult(kt_rest, 0, R_CHUNK - MINI, sc1, MINI)
        for il in range(R_CHUNK - MINI):
            _act_reduce(pr1, il, 0, ROWS_PER_CORE + MINI + il)
        for c in range(C):
            _load_shiftc(c, 1)

        def _split_block(sc, c, use_act):
            if use_act:
                pm = _tt_mult(kt_mini, 0, MINI, sc, 0)
                for il in range(MINI):
                    _act_reduce(pm, il, 0, c * ROWS_PER_CORE + il)
                p2 = _tt_mult(kt_rest, 0, R_CHUNK - MINI, sc, MINI)
                for il in range(R_CHUNK - MINI):
                    _act_reduce(p2, il, 0, c * ROWS_PER_CORE + MINI + il)
            else:
                for il in range(MINI):
                    _stt(kt_mini, il, sc, il, 0, c * ROWS_PER_CORE + il)
                for il in range(MINI, R_CHUNK):
                    _stt(kt_rest, il - MINI, sc, il, 0, c * ROWS_PER_CORE + il)

        _split_block(sc2, 2, act_flag[2])
        for il in range(MINI):
            _stt(kt_mini, il, sc1, il, 0, ROWS_PER_CORE + il)

        # Per-block ACT row counts: flagged blocks are full (8), except the
        # last flagged one (6) to fine-trim the engine balance.
        act_n = [8 if f else 0 for f in act_flag]
        act_n[max(t for t in range(N_BLK) if act_flag[t])] = 6

        # --- main loop: jb-major so jb0's output DMA overlaps jb1 compute --
        for jb in range(2):
            for iblk in range(N_IBLK):
                if jb == 0 and iblk == 0:
                    continue
                kt = _load_rows(iblk, jb, 0, R_CHUNK)
                t0 = (jb * N_IBLK + iblk) * C
                # ACT blocks first so the Activation engine stays fed.
                order = sorted(range(C), key=lambda c: not act_flag[t0 + c])
                for c in order:
                    _block(kt, shiftc[(c, jb)], iblk, jb, c, act_n[t0 + c])
            nc.sync.dma_start(
                out=AP(
                    outd,
                    jb * 128 * (C * ROWS_PER_CORE),
                    [(C * ROWS_PER_CORE, 128), (1, C * ROWS_PER_CORE)],
                ),
                in_=outt[jb][:, :],
            )

    nc.compile()
    return nc


def _program():
    if "nc" not in _CACHE:
        _CACHE["nc"] = _build_program()
    return _CACHE["nc"]


def _shard_inputs(input, kernel):
    inp = np.ascontiguousarray(np.asarray(input, dtype=np.float32))
    kern = np.asarray(kernel, dtype=np.float32)
    pad = np.pad(inp, ((0, 0), (0, 0), (PAD, PAD), (PAD, PAD)), mode="reflect")
    # sliding horizontal strips: strips[b, c, r, j, kj] = pad[b, c, r, j + kj]
    strips = np.lib.stride_tricks.sliding_window_view(pad, L, axis=3)
    in_maps = []
    for core in range(N_CORES):
        b, q = divmod(core, 4)
        r0 = q * ROWS_PER_CORE
        # SHIFTC[jb, c, j, 19*r + kj] = pad[b, c, r0 + r, jb*128 + j + kj]
        s = strips[b, :, r0 : r0 + PR, :, :]  # (C, PR, 256, L)
        s = s.transpose(2, 0, 1, 3).reshape(2, 128, C, PR * L)  # (jb*128+j, c, r*L+kj)
        sc = np.ascontiguousarray(s.transpose(0, 2, 1, 3)).astype(ml_dtypes.bfloat16)
        ks = kern[b].reshape(H * W, K2)[
            q * ROWS_PER_CORE * W : (q + 1) * ROWS_PER_CORE * W
        ].astype(ml_dtypes.bfloat16)
        in_maps.append({"kern": ks, "shiftc": sc})
    return in_maps


def _unshard_output(results):
    out = np.empty((B, C, H, W), dtype=np.float32)
    for core in range(N_CORES):
        b, q = divmod(core, 4)
        arr = np.asarray(results[core]["out"])  # (256, 192) = [jb*128+j, c*64+i]
        blk = arr.reshape(2, 128, C, ROWS_PER_CORE).transpose(2, 3, 0, 1)
        out[b, :, q * ROWS_PER_CORE : (q + 1) * ROWS_PER_CORE, :] = blk.reshape(
            C, ROWS_PER_CORE, W
        )
    return out


def run_sharded(inputs, **kw):
    """Run the compiled SPMD program; returns BassKernelResults (for profiling)."""
    in_maps = _shard_inputs(inputs["input"], inputs["kernel"])
    return bass_utils.run_bass_kernel_spmd(
        _program(), in_maps, core_ids=list(range(N_CORES)), **kw
    )


def kernel(input, kernel):
    res = run_sharded({"input": input, "kernel": kernel})
    return _unshard_output(res.results)
